# revision 10
# baseline (speedup 1.0000x reference)
"""Single-head attention (B=8, S=2048, E=1024, H=128) with softmax + deterministic
dropout, data-parallel over batch across 8 NeuronCores (one batch element per core).

Per-core layout strategy ("transposed attention", v3):
  - host ships xT fp16 [E, S] (for the v projection / value path) AND
    x8 fp8e4m3 [E, S] (for the q/k projections; DoubleRow perf mode)
  - host ships weights pre-chunked [128, NE*H]: fp16 wv, fp8 wq/wk scaled by
    16 (uniform(-1/32,1/32) values would land in the e4m3 subnormal range;
    the 16x is compensated in the softmax scale: SCALE/256)
  - host ships keep_g (dropout mask) [NSG, 128, NT*SG] fp16 {0,1}: per-s-group
    contiguous slabs so each group's mask is one 128x16KB DMA
  - qT/kT[h, s] = w8.T @ x8        (PE fp8 DoubleRow: 2 e-chunks per matmul,
                                    0.5 cycles/col; e-pair-major, 8 PSUM banks)
  - v natural [t, h] = x chunk.T @ wv  (PE fp16, t-major)
  - attT[t, s] pairs: two 128x512 fp16 score matmuls into one 2-bank PSUM tile
  - expT = exp(attT * SCALE/256)   (ACT, one 1024-wide op per pair tile; ACT is
                                    the attention-phase pacing engine)
  - attd = expT * keep             (DVE, 1024-wide)
  - pair-sums = expT_j + expT_{j+4} (DVE; -> [128, 4, 2, SG])
  - denom[1, s] = ones.T @ pair-sums (PE, 8 M=1 512-col matmuls; interleaved
                                    into the NEXT s-group so they never starve
                                    the score->exp chain at group boundaries)
  - outT[h, s] += v[t].T @ attd    (PE fp16, fp32 PSUM accumulation)
  - outT (f16) and denom (f32) ship to DRAM unnormalized; the host computes
    out = (outT / (0.9 * den)).T   (layout choice + cheap elementwise on host)

Precision: fp8 q/k adds ~0.7% relative error on the output through the softmax
logits (~0.007 absolute logit error, random); the fp16 value path contributes
~5e-4. Total ~0.7% against the 2e-2 gate.
"""

import sys

for _p in ("/opt/trn_rl_repo",):
    if _p not in sys.path:
        sys.path.append(_p)

import numpy as np

B, S, E, H = 8, 2048, 1024, 128
DROP_P = 0.1
P = 128
W8_SCALE = 16.0

_program_cache = {}


def _build_program(S=S, E=E):
    key = (S, E)
    if key in _program_cache:
        return _program_cache[key]
    NT = S // P   # t-chunks
    NE = E // P   # e-chunks
    SG = 512      # s-group width (one fp32 PSUM bank)
    NSG = S // SG
    NPAIR = NT // 2  # pairs of t-chunks sharing a 2-bank psum tile

    import concourse.bass as bass  # noqa: F401
    import concourse.mybir as mybir
    import concourse.tile as tile
    from concourse import bacc

    f32 = mybir.dt.float32
    f16 = mybir.dt.float16
    f8 = mybir.dt.float8e4
    Exp = mybir.ActivationFunctionType.Exp
    DR = mybir.MatmulPerfMode.DoubleRow
    SCALE = float(E) ** -0.5 / (W8_SCALE * W8_SCALE)

    nc = bacc.Bacc("TRN2", target_bir_lowering=False, debug=False)
    x8_d = nc.dram_tensor("x8", [E, S], f8, kind="ExternalInput").ap()
    xT_d = nc.dram_tensor("xT", [E, S], f16, kind="ExternalInput").ap()
    keep_d = nc.dram_tensor("keepg", [NSG, P, NT * SG], f16, kind="ExternalInput").ap()
    wq_d = nc.dram_tensor("wq", [P, NE * H], f8, kind="ExternalInput").ap()
    wk_d = nc.dram_tensor("wk", [P, NE * H], f8, kind="ExternalInput").ap()
    wv_d = nc.dram_tensor("wv", [P, NE * H], f16, kind="ExternalInput").ap()
    outT_d = nc.dram_tensor("outT", [P, S], f16, kind="ExternalOutput").ap()
    den_d = nc.dram_tensor("den", [1, S], f32, kind="ExternalOutput").ap()

    x8_r = x8_d.rearrange("(eo p) s -> p eo s", p=P)
    xT_r = xT_d.rearrange("(eo p) s -> p eo s", p=P)
    w_rs = [w.rearrange("p (eo h) -> p eo h", eo=NE) for w in (wq_d, wk_d, wv_d)]
    keep_r = keep_d.rearrange("g p (t s) -> g p t s", t=NT)

    with tile.TileContext(nc) as tc:
        with (
            tc.tile_pool(name="consts", bufs=1) as consts,
            tc.tile_pool(name="xw", bufs=1) as xw_pool,
            tc.tile_pool(name="qkv", bufs=1) as qkv_pool,
        ):
            ones_t = consts.tile([P, 1], f16)
            nc.vector.memset(ones_t, 1.0)

            # -------- weights first (small, gate the first matmuls); x8 as
            # per-e tiles so the q/k projections start as chunks land; xT f16
            # streams behind x8 for the v projection. DMA issue is spread
            # across engine queues to parallelize descriptor generation.
            w8_js = []
            for j in range(2):
                wj = xw_pool.tile([P, NE, H], f8, tag=f"w{j}", name=f"w{j}")
                w8_js.append(wj)
            wv_sb = xw_pool.tile([P, NE, H], f16, tag="wv", name="wv")
            x8_sb = xw_pool.tile([P, NE, S], f8, tag="x8", name="x8")
            nc.sync.dma_start(w8_js[0], w_rs[0])
            nc.scalar.dma_start(x8_sb[:, 0:2, :], x8_r[:, 0:2, :])
            nc.gpsimd.dma_start(w8_js[1], w_rs[1])
            nc.gpsimd.dma_start(wv_sb, w_rs[2])
            for ep in range(1, NE // 2):
                eng = (nc.scalar, nc.gpsimd, nc.sync)[ep % 3]
                eng.dma_start(
                    x8_sb[:, 2 * ep:2 * ep + 2, :], x8_r[:, 2 * ep:2 * ep + 2, :]
                )
            xT_es = []
            for e in range(NE):
                xe = xw_pool.tile([P, S], f16, tag=f"x{e}", name=f"x{e}")
                xT_es.append(xe)
                (nc.sync, nc.scalar, nc.gpsimd)[e % 3].dma_start(xe, xT_r[:, e, :])

            # -------- projections: qT/kT [H, S] f16 (fp8 DoubleRow);
            # v natural [t, h] f16 ----
            qkT_sb = qkv_pool.tile([P, 2, S], f16)  # [h, (q|k), s]
            v_sb = qkv_pool.tile([P, NT, H], f16)   # v natural: [t_in, t_chunk, h]
            with tc.tile_pool(name="proj_ps", bufs=3, space="PSUM") as proj_ps:
                # q e-pair-major: starts on the first x8 chunk pair; four
                # 512-wide column-group accumulators live at once.
                ps_qs = [proj_ps.tile([P, SG], f32, tag=f"pq{c}", name=f"pq{c}",
                                      bufs=1) for c in range(NSG)]
                for ep in range(NE // 2):
                    es = slice(2 * ep, 2 * ep + 2)
                    for c in range(NSG):
                        nc.tensor.matmul(
                            ps_qs[c],
                            w8_js[0][:, es, :],
                            x8_sb[:, es, c * SG:(c + 1) * SG],
                            start=(ep == 0),
                            stop=(ep == NE // 2 - 1),
                            perf_mode=DR,
                        )
                for c in range(NSG):
                    nc.any.tensor_copy(qkT_sb[:, 0, c * SG:(c + 1) * SG], ps_qs[c])
                # k c-major from the (by now resident) x8
                for c in range(NSG):
                    ps_k = proj_ps.tile([P, SG], f32, tag="proj", name="ps_k")
                    for ep in range(NE // 2):
                        es = slice(2 * ep, 2 * ep + 2)
                        nc.tensor.matmul(
                            ps_k,
                            w8_js[1][:, es, :],
                            x8_sb[:, es, c * SG:(c + 1) * SG],
                            start=(ep == 0),
                            stop=(ep == NE // 2 - 1),
                            perf_mode=DR,
                        )
                    nc.any.tensor_copy(qkT_sb[:, 1, c * SG:(c + 1) * SG], ps_k)
                # v directly in natural layout: out [t_in, h] per t-chunk,
                # lhsT = xT chunk slice [e_in, t], rhs = wv chunk [e_in, h]
                for t in range(NT):
                    ps_v = proj_ps.tile([P, H], f32, tag="proj", name="ps_v")
                    for e in range(NE):
                        nc.tensor.matmul(
                            ps_v,
                            xT_es[e][:, t * P:(t + 1) * P],
                            wv_sb[:, e, :],
                            start=(e == 0),
                            stop=(e == NE - 1),
                        )
                    nc.any.tensor_copy(v_sb[:, t, :], ps_v)

            # -------- main attention loop over s-groups --------
            with (
                tc.tile_pool(name="att_ps", bufs=2, space="PSUM") as att_ps,
                tc.tile_pool(name="out_ps", bufs=2, space="PSUM") as out_ps,
                tc.tile_pool(name="den_ps", bufs=2, space="PSUM") as den_ps,
                tc.tile_pool(name="keep_pool", bufs=2) as keep_pool,
                tc.tile_pool(name="sb", bufs=10) as sb_pool,
                tc.tile_pool(name="sb2", bufs=2) as sb2_pool,
            ):
                keeps = {}

                def fetch_keep(sg):
                    keeps[sg] = keep_pool.tile([P, NT, SG], f16, tag="keep",
                                               name=f"keep{sg}")
                    nc.sync.dma_start(keeps[sg], keep_r[sg])

                fetch_keep(0)
                # deferred denominator work from the previous s-group:
                # (psum_den_tile, sums_tile, s_slice) — emitted interleaved
                # into the current group so the PE den burst never starves
                # the score->exp chain.
                pending_den = [None]

                def emit_den_step(j):
                    if pending_den[0] is None:
                        return
                    p_den, p_sums, p_sl = pending_den[0]
                    nc.tensor.matmul(
                        p_den, ones_t, p_sums[:, j // 2, j % 2, :],
                        start=(j == 0),
                        stop=(j == NPAIR - 1),
                    )
                    if j == NPAIR - 1:
                        den_sb = sb2_pool.tile([1, SG], f32, tag="den_sb")
                        nc.vector.tensor_copy(den_sb, p_den)
                        nc.sync.dma_start(den_d[:, p_sl], den_sb)
                        pending_den[0] = None

                for sg in range(NSG):
                    s_lo = sg * SG
                    s_sl = slice(s_lo, s_lo + SG)
                    if sg + 1 < NSG:
                        fetch_keep(sg + 1)
                    keep_sg = keeps.pop(sg)
                    psum_out = out_ps.tile([P, SG], f32, tag="out")
                    # pair-sums of exp tiles for the denominator (j, j+4)
                    sums = sb_pool.tile([P, 4, 2, SG], f16, tag="sums",
                                        name=f"sums{sg}", bufs=2)
                    expTs = {}
                    attds = {}

                    def emit_front(j, s_sl=s_sl, keep_sg=keep_sg,
                                   expTs=expTs, attds=attds):
                        # two score matmuls into one 2-bank psum tile
                        psum_att = att_ps.tile([P, 2, SG], f32, tag="att",
                                               name=f"att{j}")
                        for i in range(2):
                            t = 2 * j + i
                            nc.tensor.matmul(
                                psum_att[:, i, :],
                                qkT_sb[:, 1, t * P:(t + 1) * P],  # kT chunk
                                qkT_sb[:, 0, s_sl],               # qT slice
                                start=True,
                                stop=True,
                            )
                        expT = sb_pool.tile([P, 2, SG], f16, tag="exp",
                                            name=f"exp{j}")
                        nc.scalar.activation(expT, psum_att, Exp, scale=SCALE)
                        attd = sb_pool.tile([P, 2, SG], f16, tag="attd",
                                            name=f"attd{j}")
                        nc.vector.tensor_mul(out=attd, in0=expT,
                                             in1=keep_sg[:, 2 * j:2 * j + 2, :])
                        expTs[j] = expT
                        attds[j] = attd

                    def emit_sum(j, sums=sums, expTs=expTs):
                        # pair-add exp(j) + exp(j+4) -> sums[:, j]
                        nc.vector.tensor_add(
                            out=sums[:, j], in0=expTs.pop(j), in1=expTs.pop(j + 4)
                        )

                    def emit_out(j, psum_out=psum_out, attds=attds):
                        attd = attds.pop(j)
                        for i in range(2):
                            t = 2 * j + i
                            nc.tensor.matmul(
                                psum_out,
                                v_sb[:, t, :],
                                attd[:, i, :],
                                start=(t == 0),
                                stop=(t == NT - 1),
                            )

                    # software pipeline: front(j), deferred den step of the
                    # previous group, out(j-1); pair-adds as operands retire
                    for j in range(NPAIR):
                        emit_front(j)
                        emit_den_step(j)
                        if j >= 1:
                            emit_out(j - 1)
                        if j >= 5:
                            emit_sum(j - 5)
                    emit_sum(3)
                    emit_out(NPAIR - 1)

                    # defer this group's denominator matmuls into the next
                    # group (drained immediately for the last one)
                    psum_den_t = den_ps.tile([P, SG], f32, tag="den")
                    pending_den[0] = (psum_den_t[0:1, :], sums, s_sl)

                    # unnormalized transposed output -> f16 staging -> DRAM
                    outT_sb = sb2_pool.tile([P, SG], f16, tag="outT")
                    nc.vector.tensor_copy(outT_sb, psum_out)
                    nc.sync.dma_start(outT_d[:, s_sl], outT_sb)

                # drain the last group's denominator
                for j in range(NPAIR):
                    emit_den_step(j)

    nc.compile()
    _program_cache[key] = nc
    return nc


def kernel(x, wq, wk, wv, drop_u):
    from concourse import bass_utils

    x = np.asarray(x)
    wq = np.asarray(wq)
    wk = np.asarray(wk)
    wv = np.asarray(wv)
    drop_u = np.asarray(drop_u)

    nc = _build_program()
    in_maps = build_in_maps(x, wq, wk, wv, drop_u)
    last_err = None
    for _attempt in range(3):
        try:
            res = bass_utils.run_bass_kernel_spmd(
                nc, in_maps, core_ids=list(range(B)), trace=False
            )
            break
        except Exception as e:  # transient device errors — retry
            last_err = e
            import time as _time

            _time.sleep(2.0)
    else:
        raise last_err
    out = np.empty((B, S, H), dtype=np.float32)
    for b in range(B):
        outT = res.results[b]["outT"].astype(np.float32)  # [H, S]
        den = np.asarray(res.results[b]["den"]).reshape(1, S).astype(np.float32)
        out[b] = (outT / ((1.0 - DROP_P) * den)).T
    return out


def _chunk_w(w, dtype):
    NE = E // P
    return np.ascontiguousarray(
        np.asarray(w).reshape(NE, P, H).transpose(1, 0, 2).reshape(P, NE * H)
    ).astype(dtype)


def build_in_maps(x, wq, wk, wv, drop_u):
    import ml_dtypes

    NT = S // P
    SG = 512
    NSG = S // SG
    f8 = ml_dtypes.float8_e4m3fn
    wq8 = _chunk_w(np.asarray(wq) * np.float32(W8_SCALE), f8)
    wk8 = _chunk_w(np.asarray(wk) * np.float32(W8_SCALE), f8)
    wv16 = _chunk_w(wv, np.float16)
    in_maps = []
    for b in range(B):
        xTf = np.ascontiguousarray(x[b].T)
        xT = xTf.astype(np.float16)
        x8 = xTf.astype(f8)
        keepT = (drop_u[b].T >= np.float32(DROP_P)).astype(np.float16)  # [t, s]
        # per-s-group contiguous slabs: [NSG, 128, NT*SG],
        # slab[sg][p][t*SG + s'] = keepT[t*128 + p, sg*SG + s']
        keep_g = np.ascontiguousarray(
            keepT.reshape(NT, P, NSG, SG).transpose(2, 1, 0, 3).reshape(
                NSG, P, NT * SG
            )
        )
        in_maps.append(
            {"xT": xT, "x8": x8, "keepg": keep_g,
             "wq": wq8, "wk": wk8, "wv": wv16}
        )
    return in_maps


# revision 13
# speedup vs baseline: 1.0887x; 1.0887x over previous
"""Single-head attention (B=8, S=2048, E=1024, H=128) with softmax + deterministic
dropout, data-parallel over batch across 8 NeuronCores (one batch element per core).

Per-core layout strategy ("transposed attention", v4):
  - host ships x ONCE, fp16, in quad-major layout xq [4][128][NE*4*128]:
    xq[qd][p][(e*4+b)*128+ti] = x[(qd*4+b)*128+ti, e*128+p].  Per-partition
    runs are 8KB -> efficient DMA descriptors, and ONE tile per quad serves
    both the q/k projections (rhs [e_in, (b,ti)]: 512-col matmuls) and the
    v projection (lhsT [e_in, ti] per block: natural [t, h] output).
  - host ships keep (dropout mask) [NSG, 128, NT*SG] fp16 {0,1} slabs,
    DMA'd in 4 slices per s-group so the first dropout multiplies never
    wait on a full 2MB transfer.
  - per quad qd (as its 1MB lands): qT/kT cols += w.T @ xq  (8+8 512-col
    fp16 matmuls), v[4 blocks] = xq.T @ wv (32 128-col matmuls)
  - attT[t, s] pairs: two 128x512 fp16 score matmuls into one 2-bank PSUM
    tile; expT = exp(attT * E^-0.5) (ACT, 1024-wide; the attention-phase
    pacing engine); attd = expT * keep (DVE, 1024-wide)
  - pair-sums = expT_j + expT_{j+4} (DVE) -> denom[1, s] = ones.T @ sums
    (PE, 8 M=1 512-col matmuls, deferred INTO the next s-group so the den
    burst never starves the score->exp chain at group boundaries)
  - outT[h, s] += v[t].T @ attd  (PE fp16, fp32 PSUM accumulation)
  - outT (f16) and denom (f32) ship to DRAM unnormalized; the host computes
    out = (outT / (0.9 * den)).T  (layout choice + cheap elementwise on host)

Precision: fp16 rounding on x/w/q/k contributes only ~3e-5 to the softmax
logits; the fp16 value path (v, exp, attd, outT staging) dominates at
~5e-4 L2 on the output, with all contractions accumulated in fp32 PSUM.
"""

import sys

for _p in ("/opt/trn_rl_repo",):
    if _p not in sys.path:
        sys.path.append(_p)

import numpy as np

B, S, E, H = 8, 2048, 1024, 128
DROP_P = 0.1
P = 128

_program_cache = {}


def _build_program(S=S, E=E):
    key = (S, E)
    if key in _program_cache:
        return _program_cache[key]
    NT = S // P   # t-chunks
    NE = E // P   # e-chunks
    SG = 512      # s-group width (one fp32 PSUM bank)
    NSG = S // SG
    NQ = NT // 4  # quads of t-chunks (= s column groups)
    NPAIR = NT // 2  # pairs of t-chunks sharing a 2-bank psum tile

    import concourse.bass as bass  # noqa: F401
    import concourse.mybir as mybir
    import concourse.tile as tile
    from concourse import bacc

    f32 = mybir.dt.float32
    f16 = mybir.dt.float16
    Exp = mybir.ActivationFunctionType.Exp
    SCALE = float(E) ** -0.5

    nc = bacc.Bacc("TRN2", target_bir_lowering=False, debug=False)
    xq_d = nc.dram_tensor("xq", [NQ, P, NE * 4 * P], f16, kind="ExternalInput").ap()
    keep_d = nc.dram_tensor("keepg", [NSG, P, NT * SG], f16, kind="ExternalInput").ap()
    wq_d = nc.dram_tensor("wq", [P, NE * H], f16, kind="ExternalInput").ap()
    wk_d = nc.dram_tensor("wk", [P, NE * H], f16, kind="ExternalInput").ap()
    wv_d = nc.dram_tensor("wv", [P, NE * H], f16, kind="ExternalInput").ap()
    outT_d = nc.dram_tensor("outT", [P, S], f16, kind="ExternalOutput").ap()
    den_d = nc.dram_tensor("den", [1, S], f32, kind="ExternalOutput").ap()

    xq_r = xq_d.rearrange("q p (e b t) -> q p e b t", e=NE, b=4)
    w_rs = [w.rearrange("p (eo h) -> p eo h", eo=NE) for w in (wq_d, wk_d, wv_d)]
    keep_r = keep_d.rearrange("g p (t s) -> g p t s", t=NT)

    with tile.TileContext(nc) as tc:
        with (
            tc.tile_pool(name="consts", bufs=1) as consts,
            tc.tile_pool(name="xw", bufs=1) as xw_pool,
            tc.tile_pool(name="qkv", bufs=1) as qkv_pool,
            tc.tile_pool(name="keep_pool", bufs=8) as keep_pool,
        ):
            ones_t = consts.tile([P, 1], f16)
            nc.vector.memset(ones_t, 1.0)

            # -------- input DMAs: one issuer (SP), priority order --------
            w_sbs = []
            for j in range(3):
                wj = xw_pool.tile([P, NE, H], f16, tag=f"w{j}", name=f"w{j}")
                w_sbs.append(wj)
            nc.sync.dma_start(w_sbs[0], w_rs[0])
            nc.sync.dma_start(w_sbs[1], w_rs[1])

            # dropout mask: 4 slices per s-group, fetched on a rolling basis
            keeps = {}  # (sg, sl) -> tile [P, 4, SG]

            def fetch_keep(sg):
                for sl in range(4):
                    kt = keep_pool.tile([P, 4, SG], f16, tag=f"keep{sl}",
                                        name=f"keep{sg}_{sl}")
                    nc.sync.dma_start(kt, keep_r[sg, :, 4 * sl:4 * sl + 4, :])
                    keeps[(sg, sl)] = kt

            # -------- projections, quad-major --------
            qkT_sb = qkv_pool.tile([P, 2, S], f16)  # [h, (q|k), s]
            v_sb = qkv_pool.tile([P, NT, H], f16)   # v natural: [t_in, t_chunk, h]
            with (
                tc.tile_pool(name="xq_pool", bufs=1) as xq_pool,
                tc.tile_pool(name="proj_ps", bufs=2, space="PSUM") as proj_ps,
            ):
                xq_sbs = []
                for qd in range(NQ):
                    xqt = xq_pool.tile([P, NE, 4, P], f16, tag="xq",
                                       name=f"xq{qd}", bufs=NQ)
                    xq_sbs.append(xqt)
                nc.sync.dma_start(xq_sbs[0], xq_r[0])
                nc.sync.dma_start(w_sbs[2], w_rs[2])
                for qd in range(1, NQ):
                    nc.sync.dma_start(xq_sbs[qd], xq_r[qd])
                fetch_keep(0)
                for qd in range(NQ):
                    xqt = xq_sbs[qd]
                    c_sl = slice(qd * SG, (qd + 1) * SG)
                    ps_q = proj_ps.tile([P, SG], f32, tag="pq", name="ps_q")
                    ps_k = proj_ps.tile([P, SG], f32, tag="pk", name="ps_k")
                    for e in range(NE):
                        nc.tensor.matmul(
                            ps_q, w_sbs[0][:, e, :], xqt[:, e, :, :],
                            start=(e == 0), stop=(e == NE - 1),
                        )
                    for e in range(NE):
                        nc.tensor.matmul(
                            ps_k, w_sbs[1][:, e, :], xqt[:, e, :, :],
                            start=(e == 0), stop=(e == NE - 1),
                        )
                    nc.any.tensor_copy(qkT_sb[:, 0, c_sl], ps_q)
                    nc.any.tensor_copy(qkT_sb[:, 1, c_sl], ps_k)
                    for b in range(4):
                        t = 4 * qd + b
                        ps_v = proj_ps.tile([P, H], f32, tag="pv", name="ps_v")
                        for e in range(NE):
                            nc.tensor.matmul(
                                ps_v, xqt[:, e, b, :], w_sbs[2][:, e, :],
                                start=(e == 0), stop=(e == NE - 1),
                            )
                        nc.any.tensor_copy(v_sb[:, t, :], ps_v)

            # -------- main attention loop over s-groups --------
            with (
                tc.tile_pool(name="att_ps", bufs=2, space="PSUM") as att_ps,
                tc.tile_pool(name="out_ps", bufs=2, space="PSUM") as out_ps,
                tc.tile_pool(name="den_ps", bufs=1, space="PSUM") as den_ps,
                tc.tile_pool(name="sb", bufs=10) as sb_pool,
                tc.tile_pool(name="sb2", bufs=2) as sb2_pool,
            ):
                # deferred denominator work from the previous s-group
                pending_den = [None]

                def emit_den_step(j):
                    if pending_den[0] is None:
                        return
                    p_den, p_sums, p_sl = pending_den[0]
                    nc.tensor.matmul(
                        p_den, ones_t, p_sums[:, j // 2, j % 2, :],
                        start=(j == 0),
                        stop=(j == NPAIR - 1),
                    )
                    if j == NPAIR - 1:
                        den_sb = sb2_pool.tile([1, SG], f32, tag="den_sb")
                        nc.vector.tensor_copy(den_sb, p_den)
                        nc.gpsimd.dma_start(den_d[:, p_sl], den_sb)
                        pending_den[0] = None

                for sg in range(NSG):
                    s_lo = sg * SG
                    s_sl = slice(s_lo, s_lo + SG)
                    if sg + 1 < NSG:
                        fetch_keep(sg + 1)
                    keep_sls = [keeps.pop((sg, sl)) for sl in range(4)]
                    psum_out = out_ps.tile([P, SG], f32, tag="out")
                    # pair-sums of exp tiles for the denominator (j, j+4)
                    sums = sb_pool.tile([P, 4, 2, SG], f16, tag="sums",
                                        name=f"sums{sg}", bufs=2)
                    expTs = {}
                    attds = {}

                    def emit_front(j, s_sl=s_sl, keep_sls=keep_sls,
                                   expTs=expTs, attds=attds):
                        # two score matmuls into one 2-bank psum tile
                        psum_att = att_ps.tile([P, 2, SG], f32, tag="att",
                                               name=f"att{j}")
                        for i in range(2):
                            t = 2 * j + i
                            nc.tensor.matmul(
                                psum_att[:, i, :],
                                qkT_sb[:, 1, t * P:(t + 1) * P],  # kT chunk
                                qkT_sb[:, 0, s_sl],               # qT slice
                                start=True,
                                stop=True,
                            )
                        expT = sb_pool.tile([P, 2, SG], f16, tag="exp",
                                            name=f"exp{j}", bufs=9)
                        nc.scalar.activation(expT, psum_att, Exp, scale=SCALE)
                        attd = sb_pool.tile([P, 2, SG], f16, tag="attd",
                                            name=f"attd{j}", bufs=4)
                        ksl = keep_sls[j // 2][:, (2 * j) % 4:(2 * j) % 4 + 2, :]
                        nc.vector.tensor_mul(out=attd, in0=expT, in1=ksl)
                        expTs[j] = expT
                        attds[j] = attd

                    def emit_sum(j, sums=sums, expTs=expTs):
                        nc.vector.tensor_add(
                            out=sums[:, j], in0=expTs.pop(j), in1=expTs.pop(j + 4)
                        )

                    def emit_out(j, psum_out=psum_out, attds=attds):
                        attd = attds.pop(j)
                        for i in range(2):
                            t = 2 * j + i
                            nc.tensor.matmul(
                                psum_out,
                                v_sb[:, t, :],
                                attd[:, i, :],
                                start=(t == 0),
                                stop=(t == NT - 1),
                            )

                    # software pipeline: front(j), deferred den step of the
                    # previous group, out(j-1); pair-adds as operands retire
                    for j in range(NPAIR):
                        emit_front(j)
                        emit_den_step(j)
                        if j >= 1:
                            emit_out(j - 1)
                        if j >= 5:
                            emit_sum(j - 5)
                    emit_sum(3)
                    emit_out(NPAIR - 1)

                    # defer this group's denominator matmuls into the next
                    # group (drained at the end for the last one)
                    psum_den_t = den_ps.tile([P, SG], f32, tag="den")
                    pending_den[0] = (psum_den_t[0:1, :], sums, s_sl)

                    # unnormalized transposed output -> f16 staging -> DRAM
                    outT_sb = sb2_pool.tile([P, SG], f16, tag="outT")
                    nc.vector.tensor_copy(outT_sb, psum_out)
                    nc.gpsimd.dma_start(outT_d[:, s_sl], outT_sb)

                # drain the last group's denominator
                for j in range(NPAIR):
                    emit_den_step(j)

    nc.compile()
    _program_cache[key] = nc
    return nc


def kernel(x, wq, wk, wv, drop_u):
    from concourse import bass_utils

    x = np.asarray(x)
    wq = np.asarray(wq)
    wk = np.asarray(wk)
    wv = np.asarray(wv)
    drop_u = np.asarray(drop_u)

    nc = _build_program()
    in_maps = build_in_maps(x, wq, wk, wv, drop_u)
    last_err = None
    for _attempt in range(3):
        try:
            res = bass_utils.run_bass_kernel_spmd(
                nc, in_maps, core_ids=list(range(B)), trace=False
            )
            break
        except Exception as e:  # transient device errors — retry
            last_err = e
            import time as _time

            _time.sleep(2.0)
    else:
        raise last_err
    out = np.empty((B, S, H), dtype=np.float32)
    for b in range(B):
        outT = res.results[b]["outT"].astype(np.float32)  # [H, S]
        den = np.asarray(res.results[b]["den"]).reshape(1, S).astype(np.float32)
        out[b] = (outT / ((1.0 - DROP_P) * den)).T
    return out


def _chunk_w(w, dtype):
    NE = E // P
    return np.ascontiguousarray(
        np.asarray(w).reshape(NE, P, H).transpose(1, 0, 2).reshape(P, NE * H)
    ).astype(dtype)


def build_in_maps(x, wq, wk, wv, drop_u):
    NT = S // P
    NE = E // P
    SG = 512
    NSG = S // SG
    NQ = NT // 4
    wq16 = _chunk_w(wq, np.float16)
    wk16 = _chunk_w(wk, np.float16)
    wv16 = _chunk_w(wv, np.float16)
    in_maps = []
    for b in range(B):
        # xq[qd][p][(e*4+b')*128+ti] = x[(qd*4+b')*128+ti, e*128+p]
        xq = np.ascontiguousarray(
            np.asarray(x[b]).reshape(NQ, 4, P, NE, P)  # [qd, b', ti, e, p]
            .transpose(0, 4, 3, 1, 2)                  # [qd, p, e, b', ti]
            .reshape(NQ, P, NE * 4 * P)
        ).astype(np.float16)
        keepT = (drop_u[b].T >= np.float32(DROP_P)).astype(np.float16)  # [t, s]
        keep_g = np.ascontiguousarray(
            keepT.reshape(NT, P, NSG, SG).transpose(2, 1, 0, 3).reshape(
                NSG, P, NT * SG
            )
        )
        in_maps.append(
            {"xq": xq, "keepg": keep_g, "wq": wq16, "wk": wk16, "wv": wv16}
        )
    return in_maps


# revision 14
# speedup vs baseline: 1.0895x; 1.0008x over previous
"""Single-head attention (B=8, S=2048, E=1024, H=128) with softmax + deterministic
dropout, data-parallel over batch across 8 NeuronCores (one batch element per core).

Per-core layout strategy ("transposed attention", v4):
  - host ships x ONCE, fp16, in quad-major layout xq [4][128][NE*4*128]:
    xq[qd][p][(e*4+b)*128+ti] = x[(qd*4+b)*128+ti, e*128+p].  Per-partition
    runs are 8KB -> efficient DMA descriptors, and ONE tile per quad serves
    both the q/k projections (rhs [e_in, (b,ti)]: 512-col matmuls) and the
    v projection (lhsT [e_in, ti] per block: natural [t, h] output).
  - host ships keep (dropout mask) [NSG, 128, NT*SG] fp16 {0,1} slabs,
    DMA'd in 4 slices per s-group so the first dropout multiplies never
    wait on a full 2MB transfer.
  - per quad qd (as its 1MB lands): qT/kT cols += w.T @ xq  (8+8 512-col
    fp16 matmuls), v[4 blocks] = xq.T @ wv (32 128-col matmuls)
  - attT[t, s] pairs: two 128x512 fp16 score matmuls into one 2-bank PSUM
    tile; expT = exp(attT * E^-0.5) (ACT, 1024-wide; the attention-phase
    pacing engine); attd = expT * keep (DVE, 1024-wide)
  - pair-sums = expT_j + expT_{j+4} (DVE) -> denom[1, s] = ones.T @ sums
    (PE, 8 M=1 512-col matmuls, deferred INTO the next s-group so the den
    burst never starves the score->exp chain at group boundaries)
  - outT[h, s] += v[t].T @ attd  (PE fp16, fp32 PSUM accumulation)
  - outT (f16) and denom (f32) ship to DRAM unnormalized; the host computes
    out = (outT / (0.9 * den)).T  (layout choice + cheap elementwise on host)

Precision: fp16 rounding on x/w/q/k contributes only ~3e-5 to the softmax
logits; the fp16 value path (v, exp, attd, outT staging) dominates at
~5e-4 L2 on the output, with all contractions accumulated in fp32 PSUM.
"""

import sys

for _p in ("/opt/trn_rl_repo",):
    if _p not in sys.path:
        sys.path.append(_p)

import numpy as np

B, S, E, H = 8, 2048, 1024, 128
DROP_P = 0.1
P = 128

_program_cache = {}


def _build_program(S=S, E=E):
    key = (S, E)
    if key in _program_cache:
        return _program_cache[key]
    NT = S // P   # t-chunks
    NE = E // P   # e-chunks
    SG = 512      # s-group width (one fp32 PSUM bank)
    NSG = S // SG
    NQ = NT // 4  # quads of t-chunks (= s column groups)
    NPAIR = NT // 2  # pairs of t-chunks sharing a 2-bank psum tile

    import concourse.bass as bass  # noqa: F401
    import concourse.mybir as mybir
    import concourse.tile as tile
    from concourse import bacc

    f32 = mybir.dt.float32
    f16 = mybir.dt.float16
    Exp = mybir.ActivationFunctionType.Exp
    SCALE = float(E) ** -0.5

    nc = bacc.Bacc("TRN2", target_bir_lowering=False, debug=False)
    xq_d = nc.dram_tensor("xq", [NQ, P, NE * 4 * P], f16, kind="ExternalInput").ap()
    keep_d = nc.dram_tensor("keepg", [NSG, P, NT * SG], f16, kind="ExternalInput").ap()
    wall_d = nc.dram_tensor("wall", [P, 3 * NE * H], f16, kind="ExternalInput").ap()
    outT_d = nc.dram_tensor("outT", [P, S], f16, kind="ExternalOutput").ap()
    den_d = nc.dram_tensor("den", [1, S], f32, kind="ExternalOutput").ap()

    xq_r = xq_d.rearrange("q p (e b t) -> q p e b t", e=NE, b=4)
    wall_r = wall_d.rearrange("p (j eo h) -> p j eo h", j=3, eo=NE)
    keep_r = keep_d.rearrange("g p (t s) -> g p t s", t=NT)

    with tile.TileContext(nc) as tc:
        with (
            tc.tile_pool(name="consts", bufs=1) as consts,
            tc.tile_pool(name="xw", bufs=1) as xw_pool,
            tc.tile_pool(name="qkv", bufs=1) as qkv_pool,
            tc.tile_pool(name="keep_pool", bufs=8) as keep_pool,
        ):
            ones_t = consts.tile([P, 1], f16)
            nc.vector.memset(ones_t, 1.0)
            warm_sb = consts.tile([P, P], f16)
            nc.vector.memset(warm_sb, 0.0)

            # -------- input DMAs: one issuer (SP), priority order --------
            wall_sb = xw_pool.tile([P, 3, NE, H], f16, tag="wall", name="wall")
            w_sbs = [wall_sb[:, j] for j in range(3)]
            nc.sync.dma_start(wall_sb, wall_r)

            # dropout mask: 4 slices per s-group, fetched on a rolling basis
            keeps = {}  # (sg, sl) -> tile [P, 4, SG]

            def fetch_keep(sg):
                for sl in range(4):
                    kt = keep_pool.tile([P, 4, SG], f16, tag=f"keep{sl}",
                                        name=f"keep{sg}_{sl}")
                    nc.sync.dma_start(kt, keep_r[sg, :, 4 * sl:4 * sl + 4, :])
                    keeps[(sg, sl)] = kt

            # -------- projections, quad-major --------
            qkT_sb = qkv_pool.tile([P, 2, S], f16)  # [h, (q|k), s]
            v_sb = qkv_pool.tile([P, NT, H], f16)   # v natural: [t_in, t_chunk, h]
            with (
                tc.tile_pool(name="xq_pool", bufs=1) as xq_pool,
                tc.tile_pool(name="proj_ps", bufs=2, space="PSUM") as proj_ps,
            ):
                # PE clock warm-up: keep the array busy during the DMA
                # lead-in so the quad matmuls run at the ramped p-state
                ps_w = proj_ps.tile([P, P], f32, tag="pv", name="ps_warm")
                for i in range(28):
                    nc.tensor.matmul(ps_w, warm_sb, warm_sb,
                                     start=(i == 0), stop=(i == 27))
                xq_sbs = []
                for qd in range(NQ):
                    xqt = xq_pool.tile([P, NE, 4, P], f16, tag="xq",
                                       name=f"xq{qd}", bufs=NQ)
                    xq_sbs.append(xqt)
                for qd in range(NQ):
                    nc.sync.dma_start(xq_sbs[qd], xq_r[qd])
                fetch_keep(0)
                for qd in range(NQ):
                    xqt = xq_sbs[qd]
                    c_sl = slice(qd * SG, (qd + 1) * SG)
                    ps_q = proj_ps.tile([P, SG], f32, tag="pq", name="ps_q")
                    ps_k = proj_ps.tile([P, SG], f32, tag="pk", name="ps_k")
                    for e in range(NE):
                        nc.tensor.matmul(
                            ps_q, w_sbs[0][:, e, :], xqt[:, e, :, :],
                            start=(e == 0), stop=(e == NE - 1),
                        )
                    for e in range(NE):
                        nc.tensor.matmul(
                            ps_k, w_sbs[1][:, e, :], xqt[:, e, :, :],
                            start=(e == 0), stop=(e == NE - 1),
                        )
                    nc.any.tensor_copy(qkT_sb[:, 0, c_sl], ps_q)
                    nc.any.tensor_copy(qkT_sb[:, 1, c_sl], ps_k)
                    for b in range(4):
                        t = 4 * qd + b
                        ps_v = proj_ps.tile([P, H], f32, tag="pv", name="ps_v")
                        for e in range(NE):
                            nc.tensor.matmul(
                                ps_v, xqt[:, e, b, :], w_sbs[2][:, e, :],
                                start=(e == 0), stop=(e == NE - 1),
                            )
                        nc.any.tensor_copy(v_sb[:, t, :], ps_v)

            # -------- main attention loop over s-groups --------
            with (
                tc.tile_pool(name="att_ps", bufs=2, space="PSUM") as att_ps,
                tc.tile_pool(name="out_ps", bufs=2, space="PSUM") as out_ps,
                tc.tile_pool(name="den_ps", bufs=2, space="PSUM") as den_ps,
                tc.tile_pool(name="sb", bufs=10) as sb_pool,
                tc.tile_pool(name="sb2", bufs=2) as sb2_pool,
            ):
                for sg in range(NSG):
                    s_lo = sg * SG
                    s_sl = slice(s_lo, s_lo + SG)
                    if sg + 1 < NSG:
                        fetch_keep(sg + 1)
                    keep_sls = [keeps.pop((sg, sl)) for sl in range(4)]
                    psum_out = out_ps.tile([P, SG], f32, tag="out")
                    psum_den_t = den_ps.tile([P, SG], f32, tag="den")
                    psum_den = psum_den_t[0:1, :]
                    # adjacent pair-sums of exp tiles for the denominator
                    sums = sb_pool.tile([P, 4, 2, SG], f16, tag="sums",
                                        name=f"sums{sg}", bufs=2)
                    expTs = {}
                    attds = {}

                    def emit_front(j, s_sl=s_sl, keep_sls=keep_sls,
                                   expTs=expTs, attds=attds):
                        # two score matmuls into one 2-bank psum tile
                        psum_att = att_ps.tile([P, 2, SG], f32, tag="att",
                                               name=f"att{j}")
                        for i in range(2):
                            t = 2 * j + i
                            nc.tensor.matmul(
                                psum_att[:, i, :],
                                qkT_sb[:, 1, t * P:(t + 1) * P],  # kT chunk
                                qkT_sb[:, 0, s_sl],               # qT slice
                                start=True,
                                stop=True,
                            )
                        expT = sb_pool.tile([P, 2, SG], f16, tag="exp",
                                            name=f"exp{j}", bufs=4)
                        nc.scalar.activation(expT, psum_att, Exp, scale=SCALE)
                        attd = sb_pool.tile([P, 2, SG], f16, tag="attd",
                                            name=f"attd{j}", bufs=4)
                        ksl = keep_sls[j // 2][:, (2 * j) % 4:(2 * j) % 4 + 2, :]
                        nc.vector.tensor_mul(out=attd, in0=expT, in1=ksl)
                        expTs[j] = expT
                        attds[j] = attd


                    def emit_out(j, psum_out=psum_out, attds=attds):
                        attd = attds.pop(j)
                        for i in range(2):
                            t = 2 * j + i
                            nc.tensor.matmul(
                                psum_out,
                                v_sb[:, t, :],
                                attd[:, i, :],
                                start=(t == 0),
                                stop=(t == NT - 1),
                            )

                    # software pipeline: front(j); after each odd front the
                    # adjacent pair-sum and its two den matmuls; out(j-1)
                    for j in range(NPAIR):
                        emit_front(j)
                        if j % 2 == 1:
                            jp = j // 2
                            nc.vector.tensor_add(
                                out=sums[:, jp],
                                in0=expTs.pop(j - 1), in1=expTs.pop(j),
                            )
                            for i in range(2):
                                step = 2 * jp + i
                                nc.tensor.matmul(
                                    psum_den, ones_t, sums[:, jp, i, :],
                                    start=(step == 0),
                                    stop=(step == NPAIR - 1),
                                )
                        if j >= 1:
                            emit_out(j - 1)
                    emit_out(NPAIR - 1)

                    den_sb = sb2_pool.tile([1, SG], f32, tag="den_sb")
                    nc.vector.tensor_copy(den_sb, psum_den)
                    nc.gpsimd.dma_start(den_d[:, s_sl], den_sb)

                    # unnormalized transposed output -> f16 staging -> DRAM
                    outT_sb = sb2_pool.tile([P, SG], f16, tag="outT")
                    nc.vector.tensor_copy(outT_sb, psum_out)
                    nc.gpsimd.dma_start(outT_d[:, s_sl], outT_sb)


    nc.compile()
    _program_cache[key] = nc
    return nc


def kernel(x, wq, wk, wv, drop_u):
    from concourse import bass_utils

    x = np.asarray(x)
    wq = np.asarray(wq)
    wk = np.asarray(wk)
    wv = np.asarray(wv)
    drop_u = np.asarray(drop_u)

    nc = _build_program()
    in_maps = build_in_maps(x, wq, wk, wv, drop_u)
    last_err = None
    for _attempt in range(3):
        try:
            res = bass_utils.run_bass_kernel_spmd(
                nc, in_maps, core_ids=list(range(B)), trace=False
            )
            break
        except Exception as e:  # transient device errors — retry
            last_err = e
            import time as _time

            _time.sleep(2.0)
    else:
        raise last_err
    out = np.empty((B, S, H), dtype=np.float32)
    for b in range(B):
        outT = res.results[b]["outT"].astype(np.float32)  # [H, S]
        den = np.asarray(res.results[b]["den"]).reshape(1, S).astype(np.float32)
        out[b] = (outT / ((1.0 - DROP_P) * den)).T
    return out


def _chunk_w(w, dtype):
    NE = E // P
    return np.ascontiguousarray(
        np.asarray(w).reshape(NE, P, H).transpose(1, 0, 2).reshape(P, NE * H)
    ).astype(dtype)


def build_in_maps(x, wq, wk, wv, drop_u):
    NT = S // P
    NE = E // P
    SG = 512
    NSG = S // SG
    NQ = NT // 4
    wall = np.concatenate(
        [_chunk_w(w, np.float16) for w in (wq, wk, wv)], axis=1
    )  # [P, 3*NE*H]
    in_maps = []
    for b in range(B):
        # xq[qd][p][(e*4+b')*128+ti] = x[(qd*4+b')*128+ti, e*128+p]
        xq = np.ascontiguousarray(
            np.asarray(x[b]).reshape(NQ, 4, P, NE, P)  # [qd, b', ti, e, p]
            .transpose(0, 4, 3, 1, 2)                  # [qd, p, e, b', ti]
            .reshape(NQ, P, NE * 4 * P)
        ).astype(np.float16)
        keepT = (drop_u[b].T >= np.float32(DROP_P)).astype(np.float16)  # [t, s]
        keep_g = np.ascontiguousarray(
            keepT.reshape(NT, P, NSG, SG).transpose(2, 1, 0, 3).reshape(
                NSG, P, NT * SG
            )
        )
        in_maps.append(
            {"xq": xq, "keepg": keep_g, "wall": wall}
        )
    return in_maps
